# revision 25
# baseline (speedup 1.0000x reference)
"""Trainium2 Bass kernel for nn_DPASSMBlock (windowed attention + diagonal SSM block).

Sharding: 8 cores = 2 batches x 4 sequence chunks of 512 tokens. Each core
receives its chunk plus a 128-token halo. The halo serves two purposes:
  - windowed causal attention (WIN=128) needs the previous 127 keys/values;
  - the SSM recurrence s_t = A*s_{t-1} + u_t has |A| <= 0.1, so contributions
    from more than ~48 steps back underflow fp32 to exactly 0. Running the
    scan from zero-init over the last 64 halo tokens + own tokens reproduces
    the reference states to fp32 accuracy, with no cross-core comms.

All matmuls run in bf16 with fp32 PSUM accumulation. Activations are kept
feature-major ([D, tokens]) so both weight-stationary orientations of
nc.tensor.matmul are natural; layernorm runs token-major with PE transposes
in between.
"""

import numpy as np
import ml_dtypes

PHASE = 11  # full pipeline; lower values build partial programs for debugging

import concourse.bass as bass
import concourse.tile as tile
import concourse.mybir as mybir
from concourse.bass import ts, ds
from concourse.bass_utils import run_bass_kernel_spmd
from concourse.vector_clock import ScopedClock, VectorClock

F32 = mybir.dt.float32
BF16 = mybir.dt.bfloat16
AF = mybir.ActivationFunctionType
OP = mybir.AluOpType
AX = mybir.AxisListType

B, T, D, H, WIN, N = 2, 2048, 1024, 16, 128, 64
DH = D // H          # 64
DFF = 4 * D          # 4096
CH = 512             # own tokens per core
HALO = 128           # attention halo
TOK = HALO + CH      # 640
SSM_H = 64           # ssm halo actually used by the scan
NCORES = 8
KO = D // 128        # 8
MO = D // 128        # 8
KF = DFF // 128      # 32
RT = TOK // 128      # 5 row tiles of x
CT = CH // 128       # 4 own row tiles / query blocks
MASK_VAL = -1e30


class SafeTileContext(tile.TileContext):
    """Stock _drain_and_barrier packs every outstanding wait onto one Drain;
    current walrus rejects >1 sync wait on CTRL instructions. Emit one Drain
    per outstanding semaphore instead."""

    def _drain_and_barrier(self, tick_clock, wait_clock):
        gc = tick_clock.global_clock
        scoped = gc.items() if isinstance(gc, ScopedClock) else [(None, gc)]
        emitted = False
        for scope, vc in scoped:
            for proc in range(len(vc)):
                t = vc[proc]
                if t <= 0:
                    continue
                vc_one = VectorClock()
                vc_one.require_at_least(proc, t)
                d = self.nc.sync.drain()
                wait_clock.add_sem_waits(d.ins, ScopedClock({scope: vc_one}))
                emitted = True
        if not emitted:
            self.nc.sync.drain()
        self.nc.all_engine_barrier()
        popped = self.nc._tile_sem_poison_stack.pop()
        assert popped is self._sem_poison
        self.nc.clear_and_free_semaphores(list(self.sems.allocated().values()))
        self.nc.all_engine_barrier()


def _bcast_ap(dram_handle, parts):
    """Partition-broadcast read AP for a 1D DRAM tensor."""
    ap = dram_handle[:]
    return bass.AP(tensor=ap.tensor, offset=ap.offset, ap=[[0, parts]] + list(ap.ap))


def _legalize_waits(nc):
    """Current walrus rejects >1 sync wait on most instructions (2 on
    EventSemaphore). Move excess waits onto freshly inserted wait-only
    EventSemaphore instructions on the same engine, immediately before."""
    counter = 0
    for f in nc.m.functions:
        for bb in f.blocks:
            new = []
            changed = False
            for inst in bb.instructions:
                si = inst.sync_info
                waits = list(si.on_wait) if si is not None and si.on_wait else []
                cap = 2 if isinstance(inst, mybir.InstEventSemaphore) else 1
                if len(waits) > cap:
                    extra, keep = waits[:-cap], waits[-cap:]
                    for i in range(0, len(extra), 2):
                        es = mybir.InstEventSemaphore(
                            name=f"waitfix-{counter}", ins=[], outs=[]
                        )
                        counter += 1
                        es.engine = inst.engine
                        es.sync_info = mybir.SyncInfo(
                            on_wait=extra[i : i + 2], on_update=[]
                        )
                        nc.register_instruction(es)
                        new.append(es)
                    si.on_wait = keep
                    changed = True
                new.append(inst)
            if changed:
                bb.instructions = new
    return counter


def build_program():
    nc = bass.Bass()

    # ---- per-core DRAM I/O ----
    xc_d = nc.dram_tensor("xc", [TOK, D], F32, kind="ExternalInput")
    wq_d = nc.dram_tensor("wq", [MO, 128, D], BF16, kind="ExternalInput")
    wk_d = nc.dram_tensor("wk", [MO, 128, D], BF16, kind="ExternalInput")
    wv_d = nc.dram_tensor("wv", [D, D], BF16, kind="ExternalInput")
    wo_d = nc.dram_tensor("wo", [MO, 128, D], BF16, kind="ExternalInput")
    wg_d = nc.dram_tensor("wg", [MO, 128, D], BF16, kind="ExternalInput")
    w1_d = nc.dram_tensor("w1", [KF, 128, D], BF16, kind="ExternalInput")
    w2_d = nc.dram_tensor("w2", [DFF, D], BF16, kind="ExternalInput")
    bw_d = nc.dram_tensor("bw", [128, KO * N], BF16, kind="ExternalInput")
    cw_d = nc.dram_tensor("cw", [128, D], BF16, kind="ExternalInput")
    # per-feature vectors pre-transposed to [128, n_tiles] on host
    bq_d = nc.dram_tensor("bq", [128, MO], F32, kind="ExternalInput")
    bk_d = nc.dram_tensor("bk", [128, MO], F32, kind="ExternalInput")
    bo_d = nc.dram_tensor("bo", [128, MO], F32, kind="ExternalInput")
    bg_d = nc.dram_tensor("bg", [128, MO], F32, kind="ExternalInput")
    b1_d = nc.dram_tensor("b1", [128, KF], F32, kind="ExternalInput")
    g1_d = nc.dram_tensor("g1", [128, MO], F32, kind="ExternalInput")
    be1_d = nc.dram_tensor("be1", [128, MO], F32, kind="ExternalInput")
    g2_d = nc.dram_tensor("g2", [128, MO], F32, kind="ExternalInput")
    be2_d = nc.dram_tensor("be2", [128, MO], F32, kind="ExternalInput")
    bv_d = nc.dram_tensor("bv", [D], BF16, kind="ExternalInput")
    b2_d = nc.dram_tensor("b2", [D], F32, kind="ExternalInput")
    a_d = nc.dram_tensor("a", [N, 1], F32, kind="ExternalInput")
    masks_d = nc.dram_tensor("masks", [2, 128, 256], F32, kind="ExternalInput")
    idb_d = nc.dram_tensor("idb", [128, 128], BF16, kind="ExternalInput")
    out_d = nc.dram_tensor("out", [CH, D], F32, kind="ExternalOutput")

    with SafeTileContext(nc) as tc:
        with (
            tc.tile_pool(name="persist", bufs=1) as pp,
            tc.tile_pool(name="consts", bufs=1) as cp,
            tc.tile_pool(name="scratch", bufs=3) as sp,
            tc.tile_pool(name="scratch_big", bufs=3) as spb,
            tc.tile_pool(name="wstream", bufs=6) as wp,
            tc.tile_pool(name="w2stream", bufs=3) as wp2,
        ):
            # ---------- head-critical loads first: x chunk + LN1 consts ----------
            x_sb = [pp.tile([128, D], F32, tag=f"x{r}", name=f"x{r}") for r in range(RT)]
            x_dmas = []
            for r in range(RT):
                x_dmas.append(nc.sync.dma_start(x_sb[r][:], xc_d[ts(r, 128), :]))
            idb = cp.tile([128, 128], BF16, name="idb")
            nc.sync.dma_start(idb[:], idb_d[:, :])
            eps_t = cp.tile([128, 1], F32, name="eps_t")
            nc.vector.memset(eps_t[:], 1e-5)

            def load_bias(d, n, tag):
                t = cp.tile([128, n], F32, tag=tag, name=tag)
                nc.sync.dma_start(t[:], d[:, :])
                return t

            g1_sb = load_bias(g1_d, MO, "g1")
            be1_sb = load_bias(be1_d, MO, "be1")
            g2_sb = load_bias(g2_d, MO, "g2")
            be2_sb = load_bias(be2_d, MO, "be2")
            bq_sb = load_bias(bq_d, MO, "bq")
            bk_sb = load_bias(bk_d, MO, "bk")
            bo_sb = load_bias(bo_d, MO, "bo")
            bg_sb = load_bias(bg_d, MO, "bg")
            b1_sb = load_bias(b1_d, KF, "b1")

            mask_sb = cp.tile([128, 2, 256], F32, name="mask_sb")
            nc.sync.dma_start(mask_sb[:], masks_d[:].rearrange("i p f -> p i f"))
            bv_rep = cp.tile([128, D], BF16, tag="bv", name="bv")
            nc.sync.dma_start(bv_rep[:], _bcast_ap(bv_d, 128))
            b2_rep = cp.tile([128, D], F32, tag="b2", name="b2")
            nc.sync.dma_start(b2_rep[:], _bcast_ap(b2_d, 128))
            a_sb = cp.tile([N, 1], F32, name="a_sb")
            nc.sync.dma_start(a_sb[:], a_d[:, :])
            bw_sb = cp.tile([128, KO * N], BF16, name="bw_sb")
            nc.sync.dma_start(bw_sb[:], bw_d[:, :])
            cw_sb = cp.tile([128, D], BF16, name="cw_sb")
            nc.sync.dma_start(cw_sb[:], cw_d[:, :])

            # ---------- persistent buffers ----------
            gT = [pp.tile([128, CH], BF16, tag=f"gT{m}", name=f"gT{m}") for m in range(MO)]
            ssmT = [pp.tile([128, CH], BF16, tag=f"ssmT{m}", name=f"ssmT{m}") for m in range(MO)]
            attnT = [pp.tile([128, CH], BF16, tag=f"attnT{m}", name=f"attnT{m}") for m in range(MO)]
            aoT = [pp.tile([128, CH], BF16, tag=f"aoT{k}", name=f"aoT{k}") for k in range(KO)]
            h2T = [pp.tile([128, CH], BF16, tag=f"h2T{k}", name=f"h2T{k}") for k in range(KO)]
            midT = [pp.tile([128, CH], BF16, tag=f"midT{k}", name=f"midT{k}") for k in range(KF)]

            def layer_norm_tile(x_ap):
                """x_ap [128, D] f32 -> bf16 (x - mean) * rstd tile; the
                per-feature gamma/beta are fused into the transpose eviction."""
                stats = sp.tile([128, 2, 6], F32, tag="ln_stats", name="ln_stats")
                nc.vector.bn_stats(out=stats[:, 0, :], in_=x_ap[:, 0:512])
                nc.vector.bn_stats(out=stats[:, 1, :], in_=x_ap[:, 512:1024])
                mv = sp.tile([128, 2], F32, tag="ln_mv", name="ln_mv")
                nc.vector.bn_aggr(out=mv[:], in_=stats[:])
                std = sp.tile([128, 1], F32, tag="ln_std", name="ln_std")
                nc.scalar.activation(
                    out=std[:], in_=mv[:, 1:2], func=AF.Sqrt, bias=eps_t[:], scale=1.0
                )
                rstd = sp.tile([128, 1], F32, tag="ln_rstd", name="ln_rstd")
                nc.vector.reciprocal(out=rstd[:], in_=std[:])
                tnorm = spb.tile([128, D], BF16, tag="ln_t", name="ln_t")
                nc.vector.tensor_scalar(
                    out=tnorm[:],
                    in0=x_ap,
                    scalar1=mv[:, 0:1],
                    scalar2=rstd[:],
                    op0=OP.subtract,
                    op1=OP.mult,
                )
                return tnorm

            def ln_transpose(tnorm, dstT, r, ps_tr, g_sb, be_sb):
                """transpose tnorm into feature-major dstT, applying gamma/beta
                per-partition on the PSUM->SBUF eviction."""
                for c in range(KO):
                    ptr = ps_tr.tile([128, 128], BF16, tag="tr", name="ptr")
                    nc.tensor.transpose(ptr[:], tnorm[:, ts(c, 128)], idb[:])
                    nc.vector.tensor_scalar(
                        out=dstT[c][:, ts(r, 128)],
                        in0=ptr[:],
                        scalar1=g_sb[:, c : c + 1],
                        scalar2=be_sb[:, c : c + 1],
                        op0=OP.mult,
                        op1=OP.add,
                    )

            with tc.tile_pool(name="qkv_bufs", bufs=1) as pq:
                qT = [pq.tile([128, CH], BF16, tag=f"qT{k}", name=f"qT{k}") for k in range(KO)]
                kT = [pq.tile([128, TOK], BF16, tag=f"kT{k}", name=f"kT{k}") for k in range(KO)]
                v_sb = [pq.tile([128, D], BF16, tag=f"v{r}", name=f"v{r}") for r in range(RT)]

                with (
                    tc.tile_pool(name="h_bufs", bufs=1) as ph,
                    tc.tile_pool(name="ps_mm", bufs=5, space="PSUM") as ps_mm,
                    tc.tile_pool(name="ps_trA", bufs=2, space="PSUM") as ps_trA,
                ):
                    hT = [ph.tile([128, TOK], BF16, tag=f"hT{k}", name=f"hT{k}") for k in range(KO)]

                    # ---- LN1 + transpose to feature-major ----
                    for r in range(RT):
                        tnorm = layer_norm_tile(x_sb[r][:])
                        ln_transpose(tnorm, hT, r, ps_trA, g1_sb, be1_sb)

                    # ---- projections (feature-major outs) ----
                    first_w_dmas = []

                    def proj(w_dram, m, rhs_lo, rhs_w, out_ap, bias_sb, func):
                        w_sb = wp.tile([128, D], BF16, tag="w_proj", name="w_sb")
                        dma = nc.sync.dma_start(w_sb[:], w_dram[m])
                        if len(first_w_dmas) < 4:
                            first_w_dmas.append(dma)
                        ps = ps_mm.tile([128, 512], F32, tag="m512", name="ps")
                        for k in range(KO):
                            nc.tensor.matmul(
                                ps[:, :rhs_w],
                                lhsT=w_sb[:, ts(k, 128)],
                                rhs=hT[k][:, rhs_lo : rhs_lo + rhs_w],
                                start=(k == 0),
                                stop=(k == KO - 1),
                            )
                        if func is None:
                            nc.vector.tensor_scalar_add(
                                out=out_ap, in0=ps[:, :rhs_w], scalar1=bias_sb
                            )
                        else:
                            nc.scalar.activation(
                                out=out_ap, in_=ps[:, :rhs_w], func=func,
                                bias=bias_sb, scale=1.0,
                            )

                    for m in range(MO if PHASE >= 2 else 0):
                        proj(wq_d, m, HALO, CH, qT[m][:], bq_sb[:, m : m + 1], None)
                    for m in range(MO if PHASE >= 3 else 0):
                        proj(wg_d, m, HALO, CH, gT[m][:], bg_sb[:, m : m + 1], AF.Sigmoid)
                    for m in range(MO if PHASE >= 4 else 0):
                        for lo, w in ((0, 384), (384, 256)):
                            proj(wk_d, m, lo, w, kT[m][:, lo : lo + w],
                                 bk_sb[:, m : m + 1], None)
                    # V token-major: chunk-outer, k-outer, token-inner
                    for half in range(2 if PHASE >= 5 else 0):
                        psum_v = [
                            ps_mm.tile([128, 512], F32, tag="m512", name=f"ps_v{t5}")
                            for t5 in range(RT)
                        ]
                        for k in range(KO):
                            wv_sb = wp.tile([128, 512], BF16, tag="wv", name="wv_sb")
                            nc.sync.dma_start(
                                wv_sb[:], wv_d[ts(k, 128), ts(half, 512)]
                            )
                            for t5 in range(RT):
                                nc.tensor.matmul(
                                    psum_v[t5][:],
                                    lhsT=hT[k][:, ts(t5, 128)],
                                    rhs=wv_sb[:],
                                    start=(k == 0),
                                    stop=(k == KO - 1),
                                )
                        for t5 in range(RT):
                            nc.vector.tensor_tensor(
                                v_sb[t5][:, ts(half, 512)],
                                psum_v[t5][:],
                                bv_rep[:, ts(half, 512)],
                                OP.add,
                            )

                    # ---- SSM ----
                    u_sb = pp.tile([N, CH + SSM_H], F32, name="u_sb")
                    for lo, w in (((0, 512), (512, SSM_H)) if PHASE >= 6 else ()):
                        ps = ps_mm.tile([128, 512], F32, tag="m512", name="ps_u")
                        for k in range(KO):
                            nc.tensor.matmul(
                                ps[:N, :w],
                                lhsT=bw_sb[:, ds(k * N, N)],
                                rhs=hT[k][:, SSM_H + lo : SSM_H + lo + w],
                                start=(k == 0),
                                stop=(k == KO - 1),
                            )
                        nc.vector.tensor_copy(out=u_sb[:, lo : lo + w], in_=ps[:N, :w])
                    a_mat = pp.tile([N, CH + SSM_H], F32, name="a_mat")
                    states = pp.tile([128, CH + SSM_H], F32, name="states")
                    states_bf = pp.tile([128, CH], BF16, name="states_bf")
                    if PHASE >= 6:
                        nc.vector.tensor_copy(
                            out=a_mat[:], in_=a_sb[:, 0:1].to_broadcast((N, CH + SSM_H))
                        )
                        nc.vector.memset(states[:], 0.0)
                        nc.vector.tensor_tensor_scan(
                            out=states[:N, :],
                            data0=a_mat[:],
                            data1=u_sb[:],
                            initial=0.0,
                            op0=OP.mult,
                            op1=OP.add,
                        )
                        nc.vector.tensor_copy(out=states_bf[:], in_=states[:, SSM_H:])
                    for m in range(MO if PHASE >= 6 else 0):
                        ps = ps_mm.tile([128, 512], F32, tag="m512", name="ps_c")
                        nc.tensor.matmul(
                            ps[:], lhsT=cw_sb[:, ts(m, 128)], rhs=states_bf[:],
                            start=True, stop=True,
                        )
                        nc.vector.tensor_copy(out=ssmT[m][:], in_=ps[:])

                # weight prefetch should not crowd out the x loads
                for dma in first_w_dmas:
                    tile.add_dep_helper(
                        dma.ins, x_dmas[-1].ins, reason="x loads before w prefetch"
                    )

                # ---- attention ----
                # qb-outer, heads in groups of 8, two stages per group:
                #   A: scores -> +mask -> exp (keeps DVE free of ACT waits)
                #   B: 1/sum -> scale -> transpose P -> P@V -> evict
                with (
                    tc.tile_pool(name="ps_att", bufs=3, space="PSUM") as ps_att,
                    tc.tile_pool(name="ps_trB", bufs=2, space="PSUM") as ps_trB,
                    tc.tile_pool(name="p_pool", bufs=8) as ppf,
                ):
                    for qb in range(CT if PHASE >= 7 else 0):
                        slot = 0 if qb == 0 else 1
                        for grp in range(2):
                            p_bfs, sums = {}, {}
                            for h in range(grp * 8, grp * 8 + 8):
                                ti, r0 = h // 2, (h % 2) * DH
                                s_ps = ps_att.tile([128, 256], F32, tag="s", name="s_ps")
                                nc.tensor.matmul(
                                    s_ps[:],
                                    lhsT=qT[ti][r0 : r0 + DH, ts(qb, 128)],
                                    rhs=kT[ti][r0 : r0 + DH, ds(qb * 128, 256)],
                                    start=True,
                                    stop=True,
                                )
                                nc.vector.tensor_tensor(
                                    s_ps[:], s_ps[:], mask_sb[:, slot, :], OP.add
                                )
                                # scores are O(5): exp cannot overflow fp32,
                                # skip the max-subtraction (same softmax value)
                                p_bf = ppf.tile([128, 256], BF16, tag="p_bf", name="p_bf")
                                sumexp = ppf.tile([128, 1], F32, tag="sumexp", name="sumexp")
                                nc.scalar.activation(
                                    out=p_bf[:], in_=s_ps[:], func=AF.Exp,
                                    bias=0.0, scale=1.0, accum_out=sumexp[:],
                                )
                                p_bfs[h], sums[h] = p_bf, sumexp
                            for h in range(grp * 8, grp * 8 + 8):
                                ti, r0 = h // 2, (h % 2) * DH
                                p_bf, sumexp = p_bfs[h], sums[h]
                                rs = ppf.tile([128, 1], F32, tag="rs", name="rs")
                                nc.vector.reciprocal(out=rs[:], in_=sumexp[:])
                                nc.vector.tensor_scalar_mul(p_bf[:], p_bf[:], rs[:])
                                ao_ps = ps_att.tile([DH, 128], F32, tag="ao", name="ao_ps")
                                for kb in range(2):
                                    ptr = ps_trB.tile([128, 128], BF16, tag="tr", name="ptr")
                                    nc.tensor.transpose(
                                        ptr[:], p_bf[:, ts(kb, 128)], idb[:]
                                    )
                                    pt_sb = sp.tile([128, 128], BF16, tag="pt", name="pt_sb")
                                    nc.vector.tensor_copy(out=pt_sb[:], in_=ptr[:])
                                    nc.tensor.matmul(
                                        ao_ps[:],
                                        lhsT=v_sb[qb + kb][:, ds(h * DH, DH)],
                                        rhs=pt_sb[:],
                                        start=(kb == 0),
                                        stop=(kb == 1),
                                    )
                                nc.vector.tensor_copy(
                                    out=aoT[ti][r0 : r0 + DH, ts(qb, 128)],
                                    in_=ao_ps[:],
                                )

            # ---- WO, gated fusion, x1, LN2, h2T ----
            with (
                tc.tile_pool(name="ps_mm2", bufs=3, space="PSUM") as ps_mm2,
                tc.tile_pool(name="ps_trC", bufs=2, space="PSUM") as ps_trC,
            ):
                for m in range(MO if PHASE >= 8 else 0):
                    wo_sb = wp.tile([128, D], BF16, tag="w_proj", name="wo_sb")
                    nc.sync.dma_start(wo_sb[:], wo_d[m])
                    ps = ps_mm2.tile([128, 512], F32, tag="m512", name="ps_wo")
                    for k in range(KO):
                        nc.tensor.matmul(
                            ps[:],
                            lhsT=wo_sb[:, ts(k, 128)],
                            rhs=aoT[k][:],
                            start=(k == 0),
                            stop=(k == KO - 1),
                        )
                    nc.vector.tensor_scalar_add(
                        out=attnT[m][:], in0=ps[:], scalar1=bo_sb[:, m : m + 1]
                    )
                # delta = g*(attn - ssm) + ssm, in-place on attnT (bf16)
                for m in range(MO if PHASE >= 8 else 0):
                    nc.vector.tensor_tensor(
                        attnT[m][:], attnT[m][:], ssmT[m][:], OP.subtract
                    )
                    nc.vector.tensor_tensor(attnT[m][:], attnT[m][:], gT[m][:], OP.mult)
                    nc.vector.tensor_tensor(attnT[m][:], attnT[m][:], ssmT[m][:], OP.add)
                # x1 = x + delta^T per row tile, then LN2 + h2 transposes
                for c in range(CT if PHASE >= 8 else 0):
                    for mg in range(2):
                        ptrw = ps_trC.tile([128, 512], BF16, tag="trw", name="ptrw")
                        for mm in range(4):
                            m = mg * 4 + mm
                            nc.tensor.transpose(
                                ptrw[:, ts(mm, 128)], attnT[m][:, ts(c, 128)], idb[:]
                            )
                        nc.vector.tensor_tensor(
                            x_sb[c + 1][:, ts(mg, 512)],
                            x_sb[c + 1][:, ts(mg, 512)],
                            ptrw[:],
                            OP.add,
                        )
                    if PHASE >= 9:
                        tnorm = layer_norm_tile(x_sb[c + 1][:])
                        ln_transpose(tnorm, h2T, c, ps_trC, g2_sb, be2_sb)

            # ---- MLP ----
            with (
                tc.tile_pool(name="ps_mlp", bufs=3, space="PSUM") as ps_mlp,
                tc.tile_pool(name="ps_acc", bufs=4, space="PSUM") as ps_acc,
            ):
                for kf in range(KF if PHASE >= 10 else 0):
                    w1_sb = wp.tile([128, D], BF16, tag="w_proj", name="w1_sb")
                    nc.sync.dma_start(w1_sb[:], w1_d[kf])
                    ps = ps_mlp.tile([128, 512], F32, tag="m512", name="ps_w1")
                    for k in range(KO):
                        nc.tensor.matmul(
                            ps[:],
                            lhsT=w1_sb[:, ts(k, 128)],
                            rhs=h2T[k][:],
                            start=(k == 0),
                            stop=(k == KO - 1),
                        )
                    nc.scalar.activation(
                        out=midT[kf][:],
                        in_=ps[:],
                        func=AF.Gelu,
                        bias=b1_sb[:, kf : kf + 1],
                        scale=1.0,
                    )
                # W2 token-major with held accumulators; out = x1 + mlp + b2
                for half in range(2 if PHASE >= 11 else 0):
                    psum_o = [
                        ps_acc.tile([128, 512], F32, tag="acc", name=f"ps_o{tok}")
                        for tok in range(CT)
                    ]
                    w2r = w2_d.rearrange("(kf p) d -> p kf d", p=128)
                    for kf2 in range(KF // 2):
                        w2_sb = wp2.tile([128, 2, 512], BF16, tag="w2", name="w2_sb")
                        nc.sync.dma_start(
                            w2_sb[:],
                            w2r[:, ds(kf2 * 2, 2), ts(half, 512)],
                        )
                        for j in range(2):
                            kf = kf2 * 2 + j
                            for tok in range(CT):
                                nc.tensor.matmul(
                                    psum_o[tok][:],
                                    lhsT=midT[kf][:, ts(tok, 128)],
                                    rhs=w2_sb[:, j, :],
                                    start=(kf == 0),
                                    stop=(kf == KF - 1),
                                )
                    for tok in range(CT):
                        ot = spb.tile([128, 512], F32, tag="oacc", name="ot")
                        nc.vector.tensor_tensor(
                            ot[:], psum_o[tok][:], b2_rep[:, ts(half, 512)], OP.add
                        )
                        nc.vector.tensor_tensor(
                            ot[:], ot[:], x_sb[tok + 1][:, ts(half, 512)], OP.add
                        )
                        nc.sync.dma_start(
                            out_d[ts(tok, 128), ts(half, 512)], ot[:]
                        )

    _legalize_waits(nc)
    return nc


def _pretile(w):
    """[Din, Dout] row-major -> [Dout/128, 128, Din] with [m, p, ko*128+c] =
    w[ko*128+p, m*128+c], so each m-slice DMAs contiguously per partition."""
    din, dout = w.shape
    ko, mo = din // 128, dout // 128
    return (
        w.reshape(ko, 128, mo, 128).transpose(2, 1, 0, 3).reshape(mo, 128, din).copy()
    )


def _masks():
    r = np.arange(128)[:, None]
    c = np.arange(256)[None, :]
    rest = np.where((c > r) & (c <= r + 128), 0.0, MASK_VAL).astype(np.float32)
    first = np.where((c >= 128) & ((c - 128) <= r), 0.0, MASK_VAL).astype(np.float32)
    return first, rest


_PROGRAM = None


def shard_inputs(inputs):
    bf = ml_dtypes.bfloat16
    f32 = np.float32
    x = np.asarray(inputs["x"], f32)
    scale = np.float32(1.0 / np.sqrt(np.float32(DH)))

    def btile(b, n):
        return np.ascontiguousarray(np.asarray(b, f32).reshape(n, 128).T)

    mask_first, mask_rest = _masks()
    ident = np.eye(128)

    common = dict(
        wq=_pretile(np.asarray(inputs["WQ"], f32) * scale).astype(bf),
        wk=_pretile(np.asarray(inputs["WK"], f32)).astype(bf),
        wv=np.asarray(inputs["WV"], f32).astype(bf),
        wo=_pretile(np.asarray(inputs["WO"], f32)).astype(bf),
        wg=_pretile(np.asarray(inputs["Wg"], f32)).astype(bf),
        w1=_pretile(np.asarray(inputs["W1"], f32)).astype(bf),
        w2=np.asarray(inputs["W2"], f32).astype(bf),
        bw=np.asarray(inputs["Bw"], f32)
        .reshape(KO, 128, N)
        .transpose(1, 0, 2)
        .reshape(128, KO * N)
        .astype(bf),
        cw=np.concatenate(
            [np.asarray(inputs["Cw"], f32), np.zeros((128 - N, D), f32)], axis=0
        ).astype(bf),
        bq=btile(np.asarray(inputs["bQ"], f32) * scale, MO),
        bk=btile(inputs["bK"], MO),
        bv=np.asarray(inputs["bV"], f32).astype(bf),
        bo=btile(inputs["bO"], MO),
        bg=btile(inputs["bg"], MO),
        b1=btile(inputs["b1"], KF),
        b2=np.asarray(inputs["b2"], f32),
        g1=btile(inputs["ln1_g"], MO),
        be1=btile(inputs["ln1_b"], MO),
        g2=btile(inputs["ln2_g"], MO),
        be2=btile(inputs["ln2_b"], MO),
        a=np.asarray(inputs["A"], f32).reshape(N, 1),
        idb=ident.astype(bf),
    )

    in_maps = []
    for core in range(NCORES):
        b, j = divmod(core, 4)  # 4 chunks per batch
        s = j * CH
        xc = np.zeros((TOK, D), f32)
        if j == 0:
            xc[HALO:] = x[b, 0:CH]
        else:
            xc[:] = x[b, s - HALO : s + CH]
        m = dict(common)
        m["xc"] = xc
        m["masks"] = np.stack([mask_first if j == 0 else mask_rest, mask_rest])
        in_maps.append(m)
    return in_maps


def kernel(**inputs):
    global _PROGRAM
    if _PROGRAM is None:
        _PROGRAM = build_program()
    nc = _PROGRAM

    in_maps = shard_inputs(inputs)
    try:
        res = run_bass_kernel_spmd(nc, in_maps, list(range(NCORES)))
    except Exception:
        # transient NRT device errors have been observed; retry once
        res = run_bass_kernel_spmd(nc, in_maps, list(range(NCORES)))

    out = np.empty((B, T, D), np.float32)
    for core in range(NCORES):
        b, j = divmod(core, 4)
        out[b, j * CH : (j + 1) * CH] = res.results[core]["out"]
    return out
